# revision 10
# baseline (speedup 1.0000x reference)
"""LoRA fast-linear Trainium2 kernel.

y = x @ W.T + b + sum_l s_l * (x @ down_l.T) @ up_l.T

Host side: fold the LoRA update into the dense weight (exact by linearity),
pre-transpose/pack x per core to feature-major, cast both to bf16
(rel err ~2e-3 against the 2e-2 gate; PSUM accumulation stays fp32).
Device side: data-parallel over 8 cores; W_eff stays resident in SBUF; per
2048-token body the core streams x tiles and runs 1024 accumulating
matmuls (16 kc-chunks x 16 m-tiles x 4 out-chunks of 512) with 4 PSUM
banks per m-tile, bias-adds on the DVE during the PSUM->SBUF copy, and
stores y bf16 on the scalar-engine DMA ring (host upcasts to fp32).
"""

import sys

if "/opt/trn_rl_repo" not in sys.path:
    sys.path.insert(0, "/opt/trn_rl_repo")

import numpy as np
import ml_dtypes

BF16 = ml_dtypes.bfloat16

B, S, IN, OUT, L, R = 2, 8192, 2048, 2048, 4, 32
N_CORES = 8
TOKENS = B * S              # 16384
TOK = TOKENS // N_CORES     # 2048 tokens per core
P = 128
KC = IN // P                # 16 contraction chunks
CT = 512                    # tokens per x chunk
NCHK = TOK // CT            # 4 chunks per body
MT = CT // P                # 4 m-tiles (128 tokens) per chunk
NCH = 512                   # out-feature chunk (one fp32 PSUM bank)
NT = OUT // NCH             # 4

_NC_CACHE = {}


def _build_nc(repeat=1):
    """Build the per-core Bass program. ``repeat`` re-runs the whole body
    (same data, same outputs) — used only for device-time measurement via
    timing deltas, since axon has no NTFF profiling."""
    import concourse.bacc as bacc
    import concourse.mybir as mybir
    import concourse.tile as tile

    dt = mybir.dt
    BF = dt.bfloat16

    nc = bacc.Bacc("TRN2", target_bir_lowering=False, debug=False)
    xp = nc.dram_tensor("xp", [P, KC, TOK], BF, kind="ExternalInput")
    weff = nc.dram_tensor("weff", [P, KC, OUT], BF, kind="ExternalInput")
    bias = nc.dram_tensor("bias", [OUT], dt.float32, kind="ExternalInput")
    ys = nc.dram_tensor("ys", [TOK, OUT], BF, kind="ExternalOutput")

    with tile.TileContext(nc) as tc:
        with (
            tc.tile_pool(name="const", bufs=1) as constp,
            tc.tile_pool(name="xpool", bufs=4) as xpool,
            tc.tile_pool(name="ypool", bufs=4) as ypool,
            tc.tile_pool(name="pp_y", bufs=8, space="PSUM") as pp_y,
        ):
            # resident weights: loaded once, reused by every body
            weff_sb = constp.tile([P, KC, OUT], BF)
            q = KC // 4
            for i in range(4):
                nc.sync.dma_start(
                    weff_sb[:, i * q : (i + 1) * q, :],
                    weff.ap()[:, i * q : (i + 1) * q, :],
                )
            bias_bc = constp.tile([P, OUT], dt.float32)
            nc.sync.dma_start(bias_bc[:], bias.ap()[None, :].to_broadcast((P, OUT)))

            chunk_tiles = {}

            def load_chunk(body, c):
                t = xpool.tile([P, KC, CT], BF, tag="xc")
                nc.sync.dma_start(t[:], xp.ap()[:, :, c * CT : (c + 1) * CT])
                chunk_tiles[(body, c)] = t

            def get_chunk(body, c):
                if (body, c) not in chunk_tiles:
                    load_chunk(body, c)
                return chunk_tiles.pop((body, c))

            load_chunk(0, 0)

            for body in range(repeat):
                for c in range(NCHK):
                    xc = get_chunk(body, c)
                    # prefetch the chunk that will reuse this pool slot
                    nb, nch = body, c + 3
                    if nch >= NCHK:
                        nb, nch = body + 1, nch - NCHK
                    if nb < repeat and (nb, nch) not in chunk_tiles:
                        load_chunk(nb, nch)
                    t0 = c * CT
                    for m in range(MT):
                        pys = [
                            pp_y.tile(
                                [P, NCH], dt.float32, tag="py", name=f"py{n}"
                            )
                            for n in range(NT)
                        ]
                        for kc in range(KC):
                            lhsT = xc[:, kc, m * P : (m + 1) * P]
                            for n in range(NT):
                                nc.tensor.matmul(
                                    pys[n][:],
                                    lhsT,
                                    weff_sb[:, kc, n * NCH : (n + 1) * NCH],
                                    start=(kc == 0),
                                    stop=(kc == KC - 1),
                                )
                        for n in range(NT):
                            y_sb = ypool.tile([P, NCH], BF, tag="y")
                            nc.vector.tensor_tensor(
                                y_sb[:],
                                pys[n][:],
                                bias_bc[:, n * NCH : (n + 1) * NCH],
                                mybir.AluOpType.add,
                            )
                            # scalar-engine HWDGE: separate ring from loads
                            nc.scalar.dma_start(
                                ys.ap()[
                                    t0 + m * P : t0 + (m + 1) * P,
                                    n * NCH : (n + 1) * NCH,
                                ],
                                y_sb[:],
                            )

    nc.compile()
    return nc


def get_nc(repeat=1):
    key = ("nc", repeat)
    if key not in _NC_CACHE:
        _NC_CACHE[key] = _build_nc(repeat)
    return _NC_CACHE[key]


def make_in_maps(x, weight, bias, downs, ups, scales):
    x = np.ascontiguousarray(np.asarray(x, dtype=np.float32)).reshape(TOKENS, IN)
    weight = np.asarray(weight, dtype=np.float32)
    bias_np = np.ascontiguousarray(np.asarray(bias, dtype=np.float32))
    downs = np.asarray(downs, dtype=np.float32)
    ups = np.asarray(ups, dtype=np.float32)
    scales = np.asarray(scales, dtype=np.float32)

    # fold LoRA into the dense weight (exact):  W_eff = W + sum_l s_l up_l@down_l
    weff = weight + np.einsum("l,lor,lri->oi", scales, ups, downs).astype(np.float32)
    # pack W_eff.T feature-major: weff_p[p, kc, o] = W_eff[o, kc*128+p]
    weff_p = np.ascontiguousarray(
        weff.T.reshape(KC, P, OUT).transpose(1, 0, 2).astype(BF16)
    )

    xb = x.astype(BF16)  # cast before transpose: half the bytes to move

    in_maps = []
    for c in range(N_CORES):
        xc = xb[c * TOK : (c + 1) * TOK]  # [TOK, IN]
        # xp[p, kc, t] = x[t, kc*128+p]
        xp_np = np.ascontiguousarray(xc.reshape(TOK, KC, P).transpose(2, 1, 0))
        in_maps.append({"xp": xp_np, "weff": weff_p, "bias": bias_np})
    return in_maps


def kernel(x, weight, bias, downs, ups, scales):
    from concourse.bass_utils import run_bass_kernel_spmd

    nc = get_nc()
    in_maps = make_in_maps(x, weight, bias, downs, ups, scales)
    res = run_bass_kernel_spmd(
        nc, in_maps, core_ids=list(range(N_CORES)), trace=False
    )
    y = np.concatenate(
        [res.results[c]["ys"].astype(np.float32) for c in range(N_CORES)], axis=0
    )
    return y.reshape(B, S, OUT)


# revision 11
# speedup vs baseline: 1.0562x; 1.0562x over previous
"""LoRA fast-linear Trainium2 kernel.

y = x @ W.T + b + sum_l s_l * (x @ down_l.T) @ up_l.T

Host side: fold the LoRA update into the dense weight (exact by linearity),
pre-transpose/pack x per core to feature-major, cast both to bf16
(rel err ~2e-3 against the 2e-2 gate; PSUM accumulation stays fp32).
Device side: data-parallel over 8 cores; W_eff stays resident in SBUF; per
2048-token body the core streams x tiles and runs 1024 accumulating
matmuls (16 kc-chunks x 16 m-tiles x 4 out-chunks of 512) with 4 PSUM
banks per m-tile, bias-adds on the DVE during the PSUM->SBUF copy, and
stores y bf16 on the scalar-engine DMA ring (host upcasts to fp32).
"""

import sys

if "/opt/trn_rl_repo" not in sys.path:
    sys.path.insert(0, "/opt/trn_rl_repo")

import numpy as np
import ml_dtypes

BF16 = ml_dtypes.bfloat16

B, S, IN, OUT, L, R = 2, 8192, 2048, 2048, 4, 32
N_CORES = 8
TOKENS = B * S              # 16384
TOK = TOKENS // N_CORES     # 2048 tokens per core
P = 128
KC = IN // P                # 16 contraction chunks
CT = 512                    # tokens per x chunk
NCHK = TOK // CT            # 4 chunks per body
MT = CT // P                # 4 m-tiles (128 tokens) per chunk
NCH = 512                   # out-feature chunk (one fp32 PSUM bank)
NT = OUT // NCH             # 4

_NC_CACHE = {}


def _build_nc(repeat=1):
    """Build the per-core Bass program. ``repeat`` re-runs the whole body
    (same data, same outputs) — used only for device-time measurement via
    timing deltas, since axon has no NTFF profiling."""
    import concourse.bacc as bacc
    import concourse.mybir as mybir
    import concourse.tile as tile

    dt = mybir.dt
    BF = dt.bfloat16

    nc = bacc.Bacc("TRN2", target_bir_lowering=False, debug=False)
    xp = nc.dram_tensor("xp", [P, KC, TOK], BF, kind="ExternalInput")
    weff = nc.dram_tensor("weff", [P, KC, OUT], BF, kind="ExternalInput")
    bias = nc.dram_tensor("bias", [OUT], dt.float32, kind="ExternalInput")
    ys = nc.dram_tensor("ys", [TOK, OUT], BF, kind="ExternalOutput")

    with tile.TileContext(nc) as tc:
        with (
            tc.tile_pool(name="const", bufs=1) as constp,
            tc.tile_pool(name="xpool", bufs=4) as xpool,
            tc.tile_pool(name="ypool", bufs=4) as ypool,
            tc.tile_pool(name="pp_y", bufs=8, space="PSUM") as pp_y,
        ):
            # resident weights: loaded once, reused by every body
            weff_sb = constp.tile([P, KC, OUT], BF)
            q = KC // 4
            for i in range(4):
                nc.sync.dma_start(
                    weff_sb[:, i * q : (i + 1) * q, :],
                    weff.ap()[:, i * q : (i + 1) * q, :],
                )
            bias_bc = constp.tile([P, OUT], dt.float32)
            nc.sync.dma_start(bias_bc[:], bias.ap()[None, :].to_broadcast((P, OUT)))

            chunk_tiles = {}

            def load_chunk(body, c):
                t = xpool.tile([P, KC, CT], BF, tag="xc")
                nc.sync.dma_start(t[:], xp.ap()[:, :, c * CT : (c + 1) * CT])
                chunk_tiles[(body, c)] = t

            def get_chunk(body, c):
                if (body, c) not in chunk_tiles:
                    load_chunk(body, c)
                return chunk_tiles.pop((body, c))

            load_chunk(0, 0)

            for body in range(repeat):
                for c in range(NCHK):
                    xc = get_chunk(body, c)
                    # prefetch the chunk that will reuse this pool slot
                    nb, nch = body, c + 3
                    if nch >= NCHK:
                        nb, nch = body + 1, nch - NCHK
                    if nb < repeat and (nb, nch) not in chunk_tiles:
                        load_chunk(nb, nch)
                    t0 = c * CT
                    for m in range(MT):
                        pys = [
                            pp_y.tile(
                                [P, NCH], dt.float32, tag="py", name=f"py{n}"
                            )
                            for n in range(NT)
                        ]
                        for kc in range(KC):
                            lhsT = xc[:, kc, m * P : (m + 1) * P]
                            for n in range(NT):
                                nc.tensor.matmul(
                                    pys[n][:],
                                    lhsT,
                                    weff_sb[:, kc, n * NCH : (n + 1) * NCH],
                                    start=(kc == 0),
                                    stop=(kc == KC - 1),
                                )
                        for n in range(NT):
                            y_sb = ypool.tile([P, NCH], BF, tag="y")
                            nc.vector.tensor_tensor(
                                y_sb[:],
                                pys[n][:],
                                bias_bc[:, n * NCH : (n + 1) * NCH],
                                mybir.AluOpType.add,
                            )
                            # scalar-engine HWDGE: separate ring from loads
                            nc.scalar.dma_start(
                                ys.ap()[
                                    t0 + m * P : t0 + (m + 1) * P,
                                    n * NCH : (n + 1) * NCH,
                                ],
                                y_sb[:],
                            )

    _dedup_ldweights(nc, mybir)
    nc.compile()
    return nc


def _dedup_ldweights(nc, mybir):
    """Drop InstLdweights that reload the exact weights already in the PE
    array.  The tile lowering splits every 2-byte matmul into LDW+MM, so the
    4 out-chunk matmuls sharing one stationary x-tile reload it 4x.  The PE
    executes its instructions in block order, and only Ldweights mutates the
    weight registers, so a repeat load with no semaphore traffic is dead."""
    removed = 0
    pe = mybir.EngineType.PE
    for blk in nc.m.functions[0].blocks:
        out = []
        last_key = None
        for inst in blk.instructions:
            if isinstance(inst, mybir.InstLdweights):
                si = inst.sync_info
                clean = si is None or (not si.on_wait and not si.on_update)
                key = repr(inst.ins[0])
                if clean and key == last_key:
                    removed += 1
                    continue
                last_key = key
            elif getattr(inst, "engine", None) == pe and not isinstance(
                inst, mybir.InstMatmult
            ):
                last_key = None  # unknown PE instruction: assume it clobbers
            out.append(inst)
        blk.instructions[:] = out
    return removed


def get_nc(repeat=1):
    key = ("nc", repeat)
    if key not in _NC_CACHE:
        _NC_CACHE[key] = _build_nc(repeat)
    return _NC_CACHE[key]


def make_in_maps(x, weight, bias, downs, ups, scales):
    x = np.ascontiguousarray(np.asarray(x, dtype=np.float32)).reshape(TOKENS, IN)
    weight = np.asarray(weight, dtype=np.float32)
    bias_np = np.ascontiguousarray(np.asarray(bias, dtype=np.float32))
    downs = np.asarray(downs, dtype=np.float32)
    ups = np.asarray(ups, dtype=np.float32)
    scales = np.asarray(scales, dtype=np.float32)

    # fold LoRA into the dense weight (exact):  W_eff = W + sum_l s_l up_l@down_l
    weff = weight + np.einsum("l,lor,lri->oi", scales, ups, downs).astype(np.float32)
    # pack W_eff.T feature-major: weff_p[p, kc, o] = W_eff[o, kc*128+p]
    weff_p = np.ascontiguousarray(
        weff.T.reshape(KC, P, OUT).transpose(1, 0, 2).astype(BF16)
    )

    xb = x.astype(BF16)  # cast before transpose: half the bytes to move

    in_maps = []
    for c in range(N_CORES):
        xc = xb[c * TOK : (c + 1) * TOK]  # [TOK, IN]
        # xp[p, kc, t] = x[t, kc*128+p]
        xp_np = np.ascontiguousarray(xc.reshape(TOK, KC, P).transpose(2, 1, 0))
        in_maps.append({"xp": xp_np, "weff": weff_p, "bias": bias_np})
    return in_maps


def kernel(x, weight, bias, downs, ups, scales):
    from concourse.bass_utils import run_bass_kernel_spmd

    nc = get_nc()
    in_maps = make_in_maps(x, weight, bias, downs, ups, scales)
    res = run_bass_kernel_spmd(
        nc, in_maps, core_ids=list(range(N_CORES)), trace=False
    )
    y = np.concatenate(
        [res.results[c]["ys"].astype(np.float32) for c in range(N_CORES)], axis=0
    )
    return y.reshape(B, S, OUT)


# revision 13
# speedup vs baseline: 1.0746x; 1.0174x over previous
"""LoRA fast-linear Trainium2 kernel.

y = x @ W.T + b + sum_l s_l * (x @ down_l.T) @ up_l.T

Host side: fold the LoRA update into the dense weight (exact by linearity),
pre-transpose/pack x per core to feature-major, cast both to bf16
(rel err ~2e-3 against the 2e-2 gate; PSUM accumulation stays fp32).
Device side: data-parallel over 8 cores; W_eff stays resident in SBUF; per
2048-token body the core streams x tiles and runs 1024 accumulating
matmuls (16 kc-chunks x 16 m-tiles x 4 out-chunks of 512) with 4 PSUM
banks per m-tile, bias-adds on the DVE during the PSUM->SBUF copy, and
stores y bf16 on the scalar-engine DMA ring (host upcasts to fp32).
"""

import sys

if "/opt/trn_rl_repo" not in sys.path:
    sys.path.insert(0, "/opt/trn_rl_repo")

import numpy as np
import ml_dtypes

BF16 = ml_dtypes.bfloat16

B, S, IN, OUT, L, R = 2, 8192, 2048, 2048, 4, 32
N_CORES = 8
TOKENS = B * S              # 16384
TOK = TOKENS // N_CORES     # 2048 tokens per core
P = 128
KC = IN // P                # 16 contraction chunks
CT = 512                    # tokens per x chunk
NCHK = TOK // CT            # 4 chunks per body
MT = CT // P                # 4 m-tiles (128 tokens) per chunk
NCH = 512                   # out-feature chunk (one fp32 PSUM bank)
NT = OUT // NCH             # 4

_NC_CACHE = {}


def _build_nc(repeat=1):
    """Build the per-core Bass program. ``repeat`` re-runs the whole body
    (same data, same outputs) — used only for device-time measurement via
    timing deltas, since axon has no NTFF profiling."""
    import concourse.bacc as bacc
    import concourse.mybir as mybir
    import concourse.tile as tile

    dt = mybir.dt
    BF = dt.bfloat16

    nc = bacc.Bacc("TRN2", target_bir_lowering=False, debug=False)
    xp = nc.dram_tensor("xp", [P, KC, TOK], BF, kind="ExternalInput")
    weff = nc.dram_tensor("weff", [P, KC, OUT], BF, kind="ExternalInput")
    bias = nc.dram_tensor("bias", [OUT], dt.float32, kind="ExternalInput")
    ys = nc.dram_tensor("ys", [TOK, OUT], BF, kind="ExternalOutput")

    with tile.TileContext(nc) as tc:
        with (
            tc.tile_pool(name="const", bufs=1) as constp,
            tc.tile_pool(name="xpool", bufs=4) as xpool,
            tc.tile_pool(name="ypool", bufs=4) as ypool,
            tc.tile_pool(name="pp_y", bufs=2, space="PSUM") as pp_y,
        ):
            # resident weights: loaded once, reused by every body
            weff_sb = constp.tile([P, KC, OUT], BF)
            q = KC // 4
            for i in range(4):
                nc.sync.dma_start(
                    weff_sb[:, i * q : (i + 1) * q, :],
                    weff.ap()[:, i * q : (i + 1) * q, :],
                )
            bias_bc = constp.tile([P, OUT], dt.float32)
            nc.sync.dma_start(bias_bc[:], bias.ap()[None, :].to_broadcast((P, OUT)))

            chunk_tiles = {}

            def load_chunk(body, c):
                t = xpool.tile([P, KC, CT], BF, tag="xc")
                nc.sync.dma_start(t[:], xp.ap()[:, :, c * CT : (c + 1) * CT])
                chunk_tiles[(body, c)] = t

            def get_chunk(body, c):
                if (body, c) not in chunk_tiles:
                    load_chunk(body, c)
                return chunk_tiles.pop((body, c))

            load_chunk(0, 0)

            for body in range(repeat):
                for c in range(NCHK):
                    xc = get_chunk(body, c)
                    # prefetch the chunk that will reuse this pool slot
                    nb, nch = body, c + 3
                    if nch >= NCHK:
                        nb, nch = body + 1, nch - NCHK
                    if nb < repeat and (nb, nch) not in chunk_tiles:
                        load_chunk(nb, nch)
                    t0 = c * CT
                    for m in range(MT):
                        # one 4-bank PSUM tile per m-tile: each n-slice is one
                        # bank, drained by a single fused DVE op + one store
                        py = pp_y.tile([P, NT, NCH], dt.float32, tag="py")
                        for kc in range(KC):
                            lhsT = xc[:, kc, m * P : (m + 1) * P]
                            for n in range(NT):
                                nc.tensor.matmul(
                                    py[:, n, :],
                                    lhsT,
                                    weff_sb[:, kc, n * NCH : (n + 1) * NCH],
                                    start=(kc == 0),
                                    stop=(kc == KC - 1),
                                )
                        y_sb = ypool.tile([P, OUT], BF, tag="y")
                        nc.vector.tensor_tensor(
                            y_sb[:],
                            py[:].rearrange("p n o -> p (n o)"),
                            bias_bc[:],
                            mybir.AluOpType.add,
                        )
                        # scalar-engine HWDGE: separate ring from loads
                        nc.scalar.dma_start(
                            ys.ap()[t0 + m * P : t0 + (m + 1) * P, :],
                            y_sb[:],
                        )

    _dedup_ldweights(nc, mybir)
    nc.compile()
    return nc


def _dedup_ldweights(nc, mybir):
    """Drop InstLdweights that reload the exact weights already in the PE
    array.  The tile lowering splits every 2-byte matmul into LDW+MM, so the
    4 out-chunk matmuls sharing one stationary x-tile reload it 4x.  The PE
    executes its instructions in block order, and only Ldweights mutates the
    weight registers, so a repeat load with no semaphore traffic is dead."""
    removed = 0
    pe = mybir.EngineType.PE
    for blk in nc.m.functions[0].blocks:
        out = []
        last_key = None
        for inst in blk.instructions:
            if isinstance(inst, mybir.InstLdweights):
                si = inst.sync_info
                clean = si is None or (not si.on_wait and not si.on_update)
                key = repr(inst.ins[0])
                if clean and key == last_key:
                    removed += 1
                    continue
                last_key = key
            elif getattr(inst, "engine", None) == pe and not isinstance(
                inst, mybir.InstMatmult
            ):
                last_key = None  # unknown PE instruction: assume it clobbers
            out.append(inst)
        blk.instructions[:] = out
    return removed


def get_nc(repeat=1):
    key = ("nc", repeat)
    if key not in _NC_CACHE:
        _NC_CACHE[key] = _build_nc(repeat)
    return _NC_CACHE[key]


def make_in_maps(x, weight, bias, downs, ups, scales):
    x = np.ascontiguousarray(np.asarray(x, dtype=np.float32)).reshape(TOKENS, IN)
    weight = np.asarray(weight, dtype=np.float32)
    bias_np = np.ascontiguousarray(np.asarray(bias, dtype=np.float32))
    downs = np.asarray(downs, dtype=np.float32)
    ups = np.asarray(ups, dtype=np.float32)
    scales = np.asarray(scales, dtype=np.float32)

    # fold LoRA into the dense weight (exact):  W_eff = W + sum_l s_l up_l@down_l
    weff = weight + np.einsum("l,lor,lri->oi", scales, ups, downs).astype(np.float32)
    # pack W_eff.T feature-major: weff_p[p, kc, o] = W_eff[o, kc*128+p]
    weff_p = np.ascontiguousarray(
        weff.T.reshape(KC, P, OUT).transpose(1, 0, 2).astype(BF16)
    )

    xb = x.astype(BF16)  # cast before transpose: half the bytes to move

    in_maps = []
    for c in range(N_CORES):
        xc = xb[c * TOK : (c + 1) * TOK]  # [TOK, IN]
        # xp[p, kc, t] = x[t, kc*128+p]
        xp_np = np.ascontiguousarray(xc.reshape(TOK, KC, P).transpose(2, 1, 0))
        in_maps.append({"xp": xp_np, "weff": weff_p, "bias": bias_np})
    return in_maps


def kernel(x, weight, bias, downs, ups, scales):
    from concourse.bass_utils import run_bass_kernel_spmd

    nc = get_nc()
    in_maps = make_in_maps(x, weight, bias, downs, ups, scales)
    res = run_bass_kernel_spmd(
        nc, in_maps, core_ids=list(range(N_CORES)), trace=False
    )
    y = np.concatenate(
        [res.results[c]["ys"].astype(np.float32) for c in range(N_CORES)], axis=0
    )
    return y.reshape(B, S, OUT)
